# revision 27
# baseline (speedup 1.0000x reference)
"""Trainium2 Bass kernel for nn_CausalMoE.

Reference computation (B=2, S=2048, H=2048, G=16, GH=8, FFN=8192):
  cv        = tanh(hs @ P_extract)                        [N,G]   N = B*S = 4096
  pi        = cv @ A                                      [N,G]
  h[:,m,:]  = cv @ W1[m,:G,:] + pi[:,m,None]*W1[m,G,:] + b1[m]
  h         = gelu(h)  (exact erf gelu)                   [N,G,GH]
  effects   = sum_k h[:,m,k] W2[m,k] + b2[m]              [N,G]
  modified  = hs + 0.5 * effects @ P_route                [N,H]
  ffn_h     = gelu(modified @ ffn_w1 + ffn_b1)            [N,F]
  out       = ffn_h @ ffn_w2 + ffn_b2                     [N,H]

Strategy: pure data-parallel over the 8 NeuronCores (512 tokens/core),
weights replicated.  On-chip everything is computed feature-major
(activations stored transposed, [feature, token]) so every matmul has its
contraction dim on partitions with weights as the stationary operand.
Tokens enter/leave via PE transposes.  The two big FFN matmuls run in
float32r (rounded-fp32 PE mode, ~2e-4 matmul relerr, bf16 speed).  The
tiny causal-mechanism loop is folded into three small matmuls via
host-side weight restructuring.  FFN runs in 4 F-blocks of 2048 with an
fp32 SBUF output accumulator.  The big weights are re-tiled on the host
so every weight DMA is a single fully-contiguous 1 MiB read (8 KiB per
partition), which is what lets DMA keep up with the ~290 GB/s streaming
demand of the PE.
"""
import sys

sys.path.insert(0, "/opt/trn_rl_repo")

import numpy as np

import concourse.bacc as bacc
import concourse.mybir as mybir
import concourse.tile as tile
from concourse.bass_utils import run_bass_kernel_spmd
from concourse.masks import make_identity

F32 = mybir.dt.float32
F32R = mybir.dt.float32r
AF = mybir.ActivationFunctionType

B, S, H = 2, 2048, 2048
G, GH, F = 16, 8, 8192
N_CORES = 8
NTOK = B * S              # 4096 tokens total
T = NTOK // N_CORES       # 512 tokens per core
KO = H // 128             # 16 contraction tiles over H
FO = F // 128             # 64 F tiles
TO = T // 128             # 4 token tiles
NBLK = 4                  # F blocks
FPB = FO // NBLK          # 16 F tiles per block

_CACHE = {}


def _build():
    nc = bacc.Bacc("TRN2", target_bir_lowering=False, debug=False)
    x = nc.dram_tensor("x", [T, H], F32, kind="ExternalInput").ap()
    pe = nc.dram_tensor("pe", [128, KO, G], F32, kind="ExternalInput").ap()
    adj = nc.dram_tensor("adj", [G, G], F32, kind="ExternalInput").ap()
    w1a = nc.dram_tensor("w1a", [G, G * GH], F32, kind="ExternalInput").ap()
    w1b = nc.dram_tensor("w1b", [G, G * GH], F32, kind="ExternalInput").ap()
    b1f = nc.dram_tensor("b1f", [G * GH, 1], F32, kind="ExternalInput").ap()
    w2bd = nc.dram_tensor("w2bd", [G * GH, G], F32, kind="ExternalInput").ap()
    b2s = nc.dram_tensor("b2s", [G, 1], F32, kind="ExternalInput").ap()
    pr = nc.dram_tensor("pr", [G, H], F32, kind="ExternalInput").ap()
    # host-retiled: fw1t[fo, p, ko, f] = ffn_w1[ko*128+p, fo*128+f]
    fw1 = nc.dram_tensor("fw1", [FO, 128, KO, 128], F32, kind="ExternalInput").ap()
    fb1 = nc.dram_tensor("fb1", [128, FO], F32, kind="ExternalInput").ap()
    # host-retiled: fw2t[ho, b, p, j, h] = ffn_w2[(b*FPB+j)*128+p, ho*128+h]
    fw2 = nc.dram_tensor(
        "fw2", [KO, NBLK, 128, FPB, 128], F32, kind="ExternalInput"
    ).ap()
    fb2 = nc.dram_tensor("fb2", [128, KO], F32, kind="ExternalInput").ap()
    out = nc.dram_tensor("out", [T, H], F32, kind="ExternalOutput").ap()

    with tile.TileContext(nc) as tc:
        with (
            tc.tile_pool(name="const", bufs=1) as const,
            tc.tile_pool(name="xs", bufs=2) as xsp,
            tc.tile_pool(name="xt", bufs=1) as xtp,
            tc.tile_pool(name="h1", bufs=1) as h1p,
            tc.tile_pool(name="oacc", bufs=1) as oap,
            tc.tile_pool(name="w1", bufs=4) as w1p,
            tc.tile_pool(name="w2", bufs=5) as w2p,
            tc.tile_pool(name="sm", bufs=1) as smp,
            tc.tile_pool(name="ot", bufs=6) as otp,
            tc.tile_pool(name="mm", bufs=3, space="PSUM") as mmp,
            tc.tile_pool(name="tr", bufs=5, space="PSUM") as trp,
        ):
            # identity first on the gpsimd queue: transposes need it early.
            # An f32r copy lets transposes run in single-pass f32r mode
            # (fp32 transpose is a LOW_HIGH double pass, ~2x slower).
            ident = const.tile([128, 128], F32)
            make_identity(nc, ident[:])
            identr = const.tile([128, 128], F32R)
            nc.vector.tensor_copy(identr[:], ident[:])
            # small consts on the gpsimd DMA queue so the sync queue is
            # free for x chunks + weight streaming from t=0
            pe_sb = const.tile([128, KO, G], F32R)
            nc.gpsimd.dma_start(pe_sb[:], pe.bitcast(F32R))
            adj_sb = const.tile([G, G], F32R)
            nc.gpsimd.dma_start(adj_sb[:], adj.bitcast(F32R))
            w1a_sb = const.tile([G, G * GH], F32R)
            nc.gpsimd.dma_start(w1a_sb[:], w1a.bitcast(F32R))
            w1b_sb = const.tile([G, G * GH], F32R)
            nc.gpsimd.dma_start(w1b_sb[:], w1b.bitcast(F32R))
            b1f_sb = const.tile([G * GH, 1], F32)
            nc.gpsimd.dma_start(b1f_sb[:], b1f)
            w2bd_sb = const.tile([G * GH, G], F32R)
            nc.gpsimd.dma_start(w2bd_sb[:], w2bd.bitcast(F32R))
            b2s_sb = const.tile([G, 1], F32)
            nc.gpsimd.dma_start(b2s_sb[:], b2s)
            pr_sb = const.tile([G, H], F32R)
            nc.gpsimd.dma_start(pr_sb[:], pr.bitcast(F32R))
            fb1_sb = const.tile([128, FO], F32)
            nc.gpsimd.dma_start(fb1_sb[:], fb1)
            fb2_sb = const.tile([128, KO], F32)
            nc.gpsimd.dma_start(fb2_sb[:], fb2)

            # ---- transpose x into feature-major xT [128, KO, T] (f32r) ----
            xT = xtp.tile([128, KO, T], F32R)
            for to in range(TO):
                xs = xsp.tile([128, H], F32R, tag="xs")
                nc.sync.dma_start(
                    xs[:], x[to * 128:(to + 1) * 128, :].bitcast(F32R)
                )
                for ho in range(KO):
                    pt = trp.tile([128, 128], F32R, tag="tr")
                    nc.tensor.transpose(
                        pt[:], xs[:, ho * 128:(ho + 1) * 128], identr[:]
                    )
                    # alternate PSUM-evict engine: DVE alone (~290ns/copy)
                    # paces the whole transpose phase, ACT is idle here
                    dst = xT[:, ho, to * 128:(to + 1) * 128]
                    if ho % 2 == 0:
                        nc.vector.tensor_copy(dst, pt[:])
                    else:
                        nc.scalar.activation(dst, pt[:], AF.Copy)

            # ---- causal-variable extraction: cv^T = tanh(Pe^T @ x^T) ----
            cv_ps = mmp.tile([128, T], F32, tag="mm")
            for ko in range(KO):
                nc.tensor.matmul(
                    cv_ps[0:G, :], pe_sb[:, ko, :], xT[:, ko, :],
                    start=(ko == 0), stop=(ko == KO - 1),
                )
            cvt_sb = smp.tile([G, T], F32R, tag="cv")
            nc.scalar.activation(cvt_sb[:], cv_ps[0:G, :], AF.Tanh)

            # pi^T = A^T @ cv^T
            pi_ps = mmp.tile([128, T], F32, tag="mm")
            nc.tensor.matmul(
                pi_ps[0:G, :], adj_sb[:], cvt_sb[:], start=True, stop=True
            )
            pit_sb = smp.tile([G, T], F32R, tag="pi")
            nc.vector.tensor_copy(pit_sb[:], pi_ps[0:G, :])

            # mechanism hidden: gelu(W1a^T @ cv + W1b^T @ pi + b1)
            h_ps = mmp.tile([128, T], F32, tag="mm")
            nc.tensor.matmul(h_ps[:], w1a_sb[:], cvt_sb[:], start=True, stop=False)
            nc.tensor.matmul(h_ps[:], w1b_sb[:], pit_sb[:], start=False, stop=True)
            hm_sb = smp.tile([G * GH, T], F32R, tag="hm")
            nc.scalar.activation(hm_sb[:], h_ps[:], AF.Gelu, bias=b1f_sb[:])

            # effects*0.5 = W2bd^T @ hm + b2*0.5
            eff_ps = mmp.tile([128, T], F32, tag="mm")
            nc.tensor.matmul(
                eff_ps[0:G, :], w2bd_sb[:], hm_sb[:], start=True, stop=True
            )
            # bias-add on DVE: keeps the ACT LUT on Gelu (no table reload)
            effs_sb = smp.tile([G, T], F32R, tag="eff")
            nc.vector.tensor_scalar_add(effs_sb[:], eff_ps[0:G, :], b2s_sb[:])

            # ---- modified^T = x^T + P_route^T @ effs  (in place on xT) ----
            for ho in range(KO):
                md = mmp.tile([128, T], F32, tag="mm")
                nc.tensor.matmul(
                    md[:], pr_sb[:, ho * 128:(ho + 1) * 128], effs_sb[:],
                    start=True, stop=True,
                )
                nc.vector.tensor_add(xT[:, ho, :], xT[:, ho, :], md[:])

            # ---- FFN in 4 F-blocks, f32r SBUF accumulator for layer 2 ----
            out_acc = oap.tile([128, KO, T], F32R)

            def finalize(ho):
                # transpose this H-tile back to token-major and store.
                # high_priority biases the scheduler to interleave these
                # with the remaining FFN2 matmul groups instead of
                # deferring them all into a serialized tail chain.
                with tc.high_priority():
                    _finalize(ho)

            def _finalize(ho):
                for to in range(TO):
                    pt = trp.tile([128, 128], F32R, tag="tr")
                    nc.tensor.transpose(
                        pt[:],
                        out_acc[:, ho, to * 128:(to + 1) * 128],
                        identr[:],
                    )
                    ot = otp.tile([128, 128], F32, tag="ot")
                    if to % 2 == 0:
                        nc.scalar.activation(ot[:], pt[:], AF.Copy)
                    else:
                        nc.vector.tensor_copy(ot[:], pt[:])
                    nc.sync.dma_start(
                        out[to * 128:(to + 1) * 128, ho * 128:(ho + 1) * 128],
                        ot[:],
                    )
            for b in range(NBLK):
                h1b = h1p.tile([128, FPB, T], F32R, tag="h1")
                for j in range(FPB):
                    fo = b * FPB + j
                    wt = w1p.tile([128, KO, 128], F32R, tag="w1")
                    nc.sync.dma_start(wt[:], fw1[fo].bitcast(F32R))
                    pf = mmp.tile([128, T], F32, tag="mm")
                    for ko in range(KO):
                        nc.tensor.matmul(
                            pf[:], wt[:, ko, :], xT[:, ko, :],
                            start=(ko == 0), stop=(ko == KO - 1),
                        )
                    nc.scalar.activation(
                        h1b[:, j, :], pf[:], AF.Gelu, bias=fb1_sb[:, fo:fo + 1]
                    )
                for ho in range(KO):
                    w2t = w2p.tile([128, FPB, 128], F32R, tag="w2")
                    nc.sync.dma_start(w2t[:], fw2[ho, b].bitcast(F32R))
                    po = mmp.tile([128, T], F32, tag="mm")
                    for j in range(FPB):
                        nc.tensor.matmul(
                            po[:], w2t[:, j, :], h1b[:, j, :],
                            start=(j == 0), stop=(j == FPB - 1),
                        )
                    if b == 0:
                        nc.vector.tensor_scalar_add(
                            out_acc[:, ho, :], po[:], fb2_sb[:, ho:ho + 1]
                        )
                    else:
                        nc.vector.tensor_add(
                            out_acc[:, ho, :], out_acc[:, ho, :], po[:]
                        )
                    if b == NBLK - 1 and ho > 0:
                        # finalize the PREVIOUS H-tile: its DVE add has had a
                        # full ho-iteration to drain, so the PE transposes
                        # don't stall on the accumulator add or PSUM copies
                        finalize(ho - 1)
                if b == NBLK - 1:
                    finalize(KO - 1)

    nc.compile()
    return nc


def _prep(inputs):
    """Host-side restructuring of weights + sharding."""
    hs = np.ascontiguousarray(np.asarray(inputs["hidden_states"], np.float32))
    W1 = np.asarray(inputs["W1"], np.float32)
    b1 = np.asarray(inputs["b1"], np.float32)
    W2 = np.asarray(inputs["W2"], np.float32)
    b2 = np.asarray(inputs["b2"], np.float32)

    w1a = np.ascontiguousarray(
        W1[:, :G, :].transpose(1, 0, 2).reshape(G, G * GH)
    )
    w1b = np.zeros((G, G * GH), np.float32)
    for m in range(G):
        w1b[m, m * GH:(m + 1) * GH] = W1[m, G, :]
    b1f = b1.reshape(G * GH, 1)
    w2bd = np.zeros((G * GH, G), np.float32)
    for m in range(G):
        w2bd[m * GH:(m + 1) * GH, m] = 0.5 * W2[m, :]
    b2s = (0.5 * b2).reshape(G, 1)

    pe = np.asarray(inputs["P_extract"], np.float32)
    # pe[h, g] -> [p, ko, g] with h = ko*128 + p
    pe_t = np.ascontiguousarray(pe.reshape(KO, 128, G).transpose(1, 0, 2))

    fw1 = np.asarray(inputs["ffn_w1"], np.float32)
    # fw1[ko*128+p, fo*128+f] -> [fo, p, ko, f]
    fw1_t = np.ascontiguousarray(
        fw1.reshape(KO, 128, FO, 128).transpose(2, 1, 0, 3)
    )
    fw2 = np.asarray(inputs["ffn_w2"], np.float32)
    # fw2[(b*FPB+j)*128+p, ho*128+h] -> [ho, b, p, j, h]
    fw2_t = np.ascontiguousarray(
        fw2.reshape(NBLK, FPB, 128, KO, 128).transpose(3, 0, 2, 1, 4)
    )

    common = {
        "pe": pe_t,
        "adj": np.ascontiguousarray(np.asarray(inputs["causal_adjacency"], np.float32)),
        "w1a": w1a,
        "w1b": w1b,
        "b1f": np.ascontiguousarray(b1f),
        "w2bd": w2bd,
        "b2s": np.ascontiguousarray(b2s),
        "pr": np.ascontiguousarray(np.asarray(inputs["P_route"], np.float32)),
        "fw1": fw1_t,
        "fb1": np.ascontiguousarray(
            np.asarray(inputs["ffn_b1"], np.float32).reshape(FO, 128).T
        ),
        "fw2": fw2_t,
        "fb2": np.ascontiguousarray(
            np.asarray(inputs["ffn_b2"], np.float32).reshape(KO, 128).T
        ),
    }
    toks = hs.reshape(NTOK, H)
    in_maps = []
    for c in range(N_CORES):
        m = dict(common)
        m["x"] = np.ascontiguousarray(toks[c * T:(c + 1) * T])
        in_maps.append(m)
    return in_maps


def run(inputs, trace=False):
    """Returns (full output [B,S,H] fp32, BassKernelResults)."""
    if "nc" not in _CACHE:
        _CACHE["nc"] = _build()
    nc = _CACHE["nc"]
    in_maps = _prep(inputs)
    res = run_bass_kernel_spmd(
        nc, in_maps, core_ids=list(range(N_CORES)), trace=trace
    )
    shards = [res.results[c]["out"] for c in range(N_CORES)]
    full = np.concatenate(shards, axis=0).reshape(B, S, H)
    return full, res


def kernel(**inputs):
    full, _ = run(inputs, trace=False)
    return full


# revision 28
# speedup vs baseline: 1.0820x; 1.0820x over previous
"""Trainium2 Bass kernel for nn_CausalMoE.

Reference computation (B=2, S=2048, H=2048, G=16, GH=8, FFN=8192):
  cv        = tanh(hs @ P_extract)                        [N,G]   N = B*S = 4096
  pi        = cv @ A                                      [N,G]
  h[:,m,:]  = cv @ W1[m,:G,:] + pi[:,m,None]*W1[m,G,:] + b1[m]
  h         = gelu(h)  (exact erf gelu)                   [N,G,GH]
  effects   = sum_k h[:,m,k] W2[m,k] + b2[m]              [N,G]
  modified  = hs + 0.5 * effects @ P_route                [N,H]
  ffn_h     = gelu(modified @ ffn_w1 + ffn_b1)            [N,F]
  out       = ffn_h @ ffn_w2 + ffn_b2                     [N,H]

Strategy: pure data-parallel over the 8 NeuronCores (512 tokens/core),
weights replicated.  On-chip everything is computed feature-major
(activations stored transposed, [feature, token]) so every matmul has its
contraction dim on partitions with weights as the stationary operand.
Tokens enter/leave via PE transposes.  The two big FFN matmuls run in
float32r (rounded-fp32 PE mode, ~2e-4 matmul relerr, bf16 speed).  The
tiny causal-mechanism loop is folded into three small matmuls via
host-side weight restructuring.  FFN runs in 4 F-blocks of 2048 with an
fp32 SBUF output accumulator.  The big weights are re-tiled on the host
so every weight DMA is a single fully-contiguous 1 MiB read (8 KiB per
partition), which is what lets DMA keep up with the ~290 GB/s streaming
demand of the PE.
"""
import sys

sys.path.insert(0, "/opt/trn_rl_repo")

import numpy as np

import concourse.bacc as bacc
import concourse.mybir as mybir
import concourse.tile as tile
from concourse.bass_utils import run_bass_kernel_spmd
from concourse.masks import make_identity

F32 = mybir.dt.float32
F32R = mybir.dt.float32r
AF = mybir.ActivationFunctionType

B, S, H = 2, 2048, 2048
G, GH, F = 16, 8, 8192
N_CORES = 8
NTOK = B * S              # 4096 tokens total
T = NTOK // N_CORES       # 512 tokens per core
KO = H // 128             # 16 contraction tiles over H
FO = F // 128             # 64 F tiles
TO = T // 128             # 4 token tiles
NBLK = 4                  # F blocks
FPB = FO // NBLK          # 16 F tiles per block

_CACHE = {}


def _build():
    nc = bacc.Bacc("TRN2", target_bir_lowering=False, debug=False)
    x = nc.dram_tensor("x", [T, H], F32, kind="ExternalInput").ap()
    pe = nc.dram_tensor("pe", [128, KO, G], F32, kind="ExternalInput").ap()
    adj = nc.dram_tensor("adj", [G, G], F32, kind="ExternalInput").ap()
    w1a = nc.dram_tensor("w1a", [G, G * GH], F32, kind="ExternalInput").ap()
    w1b = nc.dram_tensor("w1b", [G, G * GH], F32, kind="ExternalInput").ap()
    b1f = nc.dram_tensor("b1f", [G * GH, 1], F32, kind="ExternalInput").ap()
    w2bd = nc.dram_tensor("w2bd", [G * GH, G], F32, kind="ExternalInput").ap()
    b2s = nc.dram_tensor("b2s", [G, 1], F32, kind="ExternalInput").ap()
    pr = nc.dram_tensor("pr", [G, H], F32, kind="ExternalInput").ap()
    # host-retiled: fw1t[fo, p, ko, f] = ffn_w1[ko*128+p, fo*128+f]
    fw1 = nc.dram_tensor("fw1", [FO, 128, KO, 128], F32, kind="ExternalInput").ap()
    fb1 = nc.dram_tensor("fb1", [128, FO], F32, kind="ExternalInput").ap()
    # host-retiled: fw2t[ho, b, p, j, h] = ffn_w2[(b*FPB+j)*128+p, ho*128+h]
    fw2 = nc.dram_tensor(
        "fw2", [KO, NBLK, 128, FPB, 128], F32, kind="ExternalInput"
    ).ap()
    fb2 = nc.dram_tensor("fb2", [128, KO], F32, kind="ExternalInput").ap()
    out = nc.dram_tensor("out", [T, H], F32, kind="ExternalOutput").ap()

    with tile.TileContext(nc) as tc:
        with (
            tc.tile_pool(name="const", bufs=1) as const,
            tc.tile_pool(name="xs", bufs=2) as xsp,
            tc.tile_pool(name="xt", bufs=1) as xtp,
            tc.tile_pool(name="h1", bufs=1) as h1p,
            tc.tile_pool(name="oacc", bufs=1) as oap,
            tc.tile_pool(name="w1", bufs=4) as w1p,
            tc.tile_pool(name="w2", bufs=5) as w2p,
            tc.tile_pool(name="sm", bufs=1) as smp,
            tc.tile_pool(name="ot", bufs=6) as otp,
            tc.tile_pool(name="mm", bufs=4, space="PSUM") as mmp,
            tc.tile_pool(name="tr", bufs=4, space="PSUM") as trp,
        ):
            # identity first on the gpsimd queue: transposes need it early.
            # An f32r copy lets transposes run in single-pass f32r mode
            # (fp32 transpose is a LOW_HIGH double pass, ~2x slower).
            ident = const.tile([128, 128], F32)
            make_identity(nc, ident[:])
            identr = const.tile([128, 128], F32R)
            nc.vector.tensor_copy(identr[:], ident[:])
            # small consts on the gpsimd DMA queue so the sync queue is
            # free for x chunks + weight streaming from t=0
            pe_sb = const.tile([128, KO, G], F32R)
            nc.gpsimd.dma_start(pe_sb[:], pe.bitcast(F32R))
            adj_sb = const.tile([G, G], F32R)
            nc.gpsimd.dma_start(adj_sb[:], adj.bitcast(F32R))
            w1a_sb = const.tile([G, G * GH], F32R)
            nc.gpsimd.dma_start(w1a_sb[:], w1a.bitcast(F32R))
            w1b_sb = const.tile([G, G * GH], F32R)
            nc.gpsimd.dma_start(w1b_sb[:], w1b.bitcast(F32R))
            b1f_sb = const.tile([G * GH, 1], F32)
            nc.gpsimd.dma_start(b1f_sb[:], b1f)
            w2bd_sb = const.tile([G * GH, G], F32R)
            nc.gpsimd.dma_start(w2bd_sb[:], w2bd.bitcast(F32R))
            b2s_sb = const.tile([G, 1], F32)
            nc.gpsimd.dma_start(b2s_sb[:], b2s)
            pr_sb = const.tile([G, H], F32R)
            nc.gpsimd.dma_start(pr_sb[:], pr.bitcast(F32R))
            fb1_sb = const.tile([128, FO], F32)
            nc.gpsimd.dma_start(fb1_sb[:], fb1)
            fb2_sb = const.tile([128, KO], F32)
            nc.gpsimd.dma_start(fb2_sb[:], fb2)

            # ---- transpose x into feature-major xT [128, KO, T] (f32r) ----
            xT = xtp.tile([128, KO, T], F32R)
            for to in range(TO):
                xs = xsp.tile([128, H], F32R, tag="xs")
                nc.sync.dma_start(
                    xs[:], x[to * 128:(to + 1) * 128, :].bitcast(F32R)
                )
                for ho in range(KO):
                    pt = trp.tile([128, 128], F32R, tag="tr")
                    nc.tensor.transpose(
                        pt[:], xs[:, ho * 128:(ho + 1) * 128], identr[:]
                    )
                    # alternate PSUM-evict engine: DVE alone (~290ns/copy)
                    # paces the whole transpose phase, ACT is idle here
                    dst = xT[:, ho, to * 128:(to + 1) * 128]
                    if ho % 2 == 0:
                        nc.vector.tensor_copy(dst, pt[:])
                    else:
                        nc.scalar.activation(dst, pt[:], AF.Copy)

            # ---- causal-variable extraction: cv^T = tanh(Pe^T @ x^T) ----
            cv_ps = mmp.tile([128, T], F32, tag="mm")
            for ko in range(KO):
                nc.tensor.matmul(
                    cv_ps[0:G, :], pe_sb[:, ko, :], xT[:, ko, :],
                    start=(ko == 0), stop=(ko == KO - 1),
                )
            cvt_sb = smp.tile([G, T], F32R, tag="cv")
            nc.scalar.activation(cvt_sb[:], cv_ps[0:G, :], AF.Tanh)

            # pi^T = A^T @ cv^T
            pi_ps = mmp.tile([128, T], F32, tag="mm")
            nc.tensor.matmul(
                pi_ps[0:G, :], adj_sb[:], cvt_sb[:], start=True, stop=True
            )
            pit_sb = smp.tile([G, T], F32R, tag="pi")
            nc.vector.tensor_copy(pit_sb[:], pi_ps[0:G, :])

            # mechanism hidden: gelu(W1a^T @ cv + W1b^T @ pi + b1)
            h_ps = mmp.tile([128, T], F32, tag="mm")
            nc.tensor.matmul(h_ps[:], w1a_sb[:], cvt_sb[:], start=True, stop=False)
            nc.tensor.matmul(h_ps[:], w1b_sb[:], pit_sb[:], start=False, stop=True)
            hm_sb = smp.tile([G * GH, T], F32R, tag="hm")
            nc.scalar.activation(hm_sb[:], h_ps[:], AF.Gelu, bias=b1f_sb[:])

            # effects*0.5 = W2bd^T @ hm + b2*0.5
            eff_ps = mmp.tile([128, T], F32, tag="mm")
            nc.tensor.matmul(
                eff_ps[0:G, :], w2bd_sb[:], hm_sb[:], start=True, stop=True
            )
            # bias-add on DVE: keeps the ACT LUT on Gelu (no table reload)
            effs_sb = smp.tile([G, T], F32R, tag="eff")
            nc.vector.tensor_scalar_add(effs_sb[:], eff_ps[0:G, :], b2s_sb[:])

            # ---- modified^T = x^T + P_route^T @ effs  (in place on xT) ----
            for ho in range(KO):
                md = mmp.tile([128, T], F32, tag="mm")
                nc.tensor.matmul(
                    md[:], pr_sb[:, ho * 128:(ho + 1) * 128], effs_sb[:],
                    start=True, stop=True,
                )
                nc.vector.tensor_add(xT[:, ho, :], xT[:, ho, :], md[:])

            # ---- FFN in 4 F-blocks, f32r SBUF accumulator for layer 2 ----
            out_acc = oap.tile([128, KO, T], F32R)

            def finalize(ho):
                # transpose this H-tile back to token-major and store
                for to in range(TO):
                    pt = trp.tile([128, 128], F32R, tag="tr")
                    nc.tensor.transpose(
                        pt[:],
                        out_acc[:, ho, to * 128:(to + 1) * 128],
                        identr[:],
                    )
                    ot = otp.tile([128, 128], F32, tag="ot")
                    nc.scalar.activation(ot[:], pt[:], AF.Copy)
                    nc.sync.dma_start(
                        out[to * 128:(to + 1) * 128, ho * 128:(ho + 1) * 128],
                        ot[:],
                    )
            for b in range(NBLK):
                h1b = h1p.tile([128, FPB, T], F32R, tag="h1")
                for j in range(FPB):
                    fo = b * FPB + j
                    wt = w1p.tile([128, KO, 128], F32R, tag="w1")
                    nc.sync.dma_start(wt[:], fw1[fo].bitcast(F32R))
                    pf = mmp.tile([128, T], F32, tag="mm")
                    for ko in range(KO):
                        nc.tensor.matmul(
                            pf[:], wt[:, ko, :], xT[:, ko, :],
                            start=(ko == 0), stop=(ko == KO - 1),
                        )
                    nc.scalar.activation(
                        h1b[:, j, :], pf[:], AF.Gelu, bias=fb1_sb[:, fo:fo + 1]
                    )
                for ho in range(KO):
                    w2t = w2p.tile([128, FPB, 128], F32R, tag="w2")
                    nc.sync.dma_start(w2t[:], fw2[ho, b].bitcast(F32R))
                    po = mmp.tile([128, T], F32, tag="mm")
                    for j in range(FPB):
                        nc.tensor.matmul(
                            po[:], w2t[:, j, :], h1b[:, j, :],
                            start=(j == 0), stop=(j == FPB - 1),
                        )
                    if b == 0:
                        nc.vector.tensor_scalar_add(
                            out_acc[:, ho, :], po[:], fb2_sb[:, ho:ho + 1]
                        )
                    else:
                        nc.vector.tensor_add(
                            out_acc[:, ho, :], out_acc[:, ho, :], po[:]
                        )
                    if b == NBLK - 1 and ho > 0:
                        # finalize the PREVIOUS H-tile: its DVE add has had a
                        # full ho-iteration to drain, so the PE transposes
                        # don't stall on the accumulator add or PSUM copies
                        finalize(ho - 1)
                if b == NBLK - 1:
                    finalize(KO - 1)

    nc.compile()
    return nc


def _prep(inputs):
    """Host-side restructuring of weights + sharding."""
    hs = np.ascontiguousarray(np.asarray(inputs["hidden_states"], np.float32))
    W1 = np.asarray(inputs["W1"], np.float32)
    b1 = np.asarray(inputs["b1"], np.float32)
    W2 = np.asarray(inputs["W2"], np.float32)
    b2 = np.asarray(inputs["b2"], np.float32)

    w1a = np.ascontiguousarray(
        W1[:, :G, :].transpose(1, 0, 2).reshape(G, G * GH)
    )
    w1b = np.zeros((G, G * GH), np.float32)
    for m in range(G):
        w1b[m, m * GH:(m + 1) * GH] = W1[m, G, :]
    b1f = b1.reshape(G * GH, 1)
    w2bd = np.zeros((G * GH, G), np.float32)
    for m in range(G):
        w2bd[m * GH:(m + 1) * GH, m] = 0.5 * W2[m, :]
    b2s = (0.5 * b2).reshape(G, 1)

    pe = np.asarray(inputs["P_extract"], np.float32)
    # pe[h, g] -> [p, ko, g] with h = ko*128 + p
    pe_t = np.ascontiguousarray(pe.reshape(KO, 128, G).transpose(1, 0, 2))

    fw1 = np.asarray(inputs["ffn_w1"], np.float32)
    # fw1[ko*128+p, fo*128+f] -> [fo, p, ko, f]
    fw1_t = np.ascontiguousarray(
        fw1.reshape(KO, 128, FO, 128).transpose(2, 1, 0, 3)
    )
    fw2 = np.asarray(inputs["ffn_w2"], np.float32)
    # fw2[(b*FPB+j)*128+p, ho*128+h] -> [ho, b, p, j, h]
    fw2_t = np.ascontiguousarray(
        fw2.reshape(NBLK, FPB, 128, KO, 128).transpose(3, 0, 2, 1, 4)
    )

    common = {
        "pe": pe_t,
        "adj": np.ascontiguousarray(np.asarray(inputs["causal_adjacency"], np.float32)),
        "w1a": w1a,
        "w1b": w1b,
        "b1f": np.ascontiguousarray(b1f),
        "w2bd": w2bd,
        "b2s": np.ascontiguousarray(b2s),
        "pr": np.ascontiguousarray(np.asarray(inputs["P_route"], np.float32)),
        "fw1": fw1_t,
        "fb1": np.ascontiguousarray(
            np.asarray(inputs["ffn_b1"], np.float32).reshape(FO, 128).T
        ),
        "fw2": fw2_t,
        "fb2": np.ascontiguousarray(
            np.asarray(inputs["ffn_b2"], np.float32).reshape(KO, 128).T
        ),
    }
    toks = hs.reshape(NTOK, H)
    in_maps = []
    for c in range(N_CORES):
        m = dict(common)
        m["x"] = np.ascontiguousarray(toks[c * T:(c + 1) * T])
        in_maps.append(m)
    return in_maps


def run(inputs, trace=False):
    """Returns (full output [B,S,H] fp32, BassKernelResults)."""
    if "nc" not in _CACHE:
        _CACHE["nc"] = _build()
    nc = _CACHE["nc"]
    in_maps = _prep(inputs)
    res = run_bass_kernel_spmd(
        nc, in_maps, core_ids=list(range(N_CORES)), trace=trace
    )
    shards = [res.results[c]["out"] for c in range(N_CORES)]
    full = np.concatenate(shards, axis=0).reshape(B, S, H)
    return full, res


def kernel(**inputs):
    full, _ = run(inputs, trace=False)
    return full
